# revision 29
# baseline (speedup 1.0000x reference)
"""DigitCapsules routing kernel for 8 Trainium2 NeuronCores.

Math: in the reference, u_hat is an explicit broadcast of u_core over the
capsule axis i, so b stays constant along i in every routing iteration,
softmax over i is exactly uniform (1/K), and the whole 3-iteration routing
collapses (exactly, in floating point too) to:

    v[b, i, :] = squash((1/576) * sum_{r,k} x2[b, r, k] * W[b, r, k, :])

broadcast over i = 0..575, where x2 = x.reshape(B, 8, 576).transpose(0, 2, 1).
The i-broadcast is pure replication, done on the host after the gather
(np.broadcast_to) - the device computes and returns only the unique
[4, 16] v rows per core.

Sharding: batch dim B=32 across 8 cores, 4 batches per core (data parallel).

Device structure:
 - wx = [W | x2] packed fp16 on host: per batch 4 full r-tiles of 128 rows
   plus a half tile; two batches share each half tile (64 partitions each)
   so there is no zero padding.  A k-diagonal mask rides along (128 cols).
   Two big input DMAs on independent descriptor-generation paths: sync
   HWDGE and gpsimd SWDGE (two HWDGE-ring DMAs start draining ~1.1us
   apart - ring generation is serialized; SWDGE is not).
 - G: batch n's contraction lands in PSUM partitions 32n..32n+8 via PE
   column-group tiling, so the batches' accumulation chains overlap on the
   PE array; two PSUM tiles (pair b0/b1, pair b2/b3) so the first pair's
   mask-multiply is not dependency-chained behind the second pair.
 - One mask-multiply per pair (fp16 out, partition-shifted DVE reads are
   fine) extracts the k-diagonal; a one-hot [128, 4] fp16 matmul gives
   T2 [4, 128] fp32, and a tiny [4, 16, 8] reduce lands T in SBUF.
 - Squash over [4, 16]: sq/norm on DVE, Sqrt on ACT, then two PARALLEL
   reciprocals - 1/(1+norm) folds into the coefficient during the Sqrt
   latency, only 1/q remains on the post-Sqrt path (Rsqrt is banned for
   accuracy).  A dummy early Sqrt hoists the 1.3us ACT table load off the
   critical path (it otherwise lands right before the first Sqrt user).
 - Output: one 256-byte DMA of v [4, 16] fp32.
 - 8 N=512 junk matmuls (operands mostly uninitialized - Tile only needs
   one writer per tile) run during the input-DMA wait to warm the PE HAM
   clock gate (~3.4us of sustained activity flips PE from 1.2 to 2.4 GHz;
   warm matmuls pace at 56ns vs 107ns cold).

Perf notes from traces (baseline 20.4us -> this kernel ~17.5-18.2us; the
run-to-run spread is +-0.4us from HAM phase and DMA jitter):
 - ~13.0us of the measured window is FIXED (measured with a do-nothing
   1-DMA-in/1-DMA-out kernel): bass init consts+barrier (~1.1us), tiny-DMA
   in (~2.0us) / out-receipt (~1.6us) latencies, and ~8.0us of teardown -
   a runtime-injected postamble clears all 254 semaphores one instruction
   at a time.  Only the ~7.4us above the floor is optimizable.
 - Input DMA sustains only ~20B/ns per SDMA engine at these sizes;
   engines 8-15 (the second SEngine half) are measurably slower.
 - Tile tracks PSUM regions by free-dim ranges only, so partition-sliced
   reads of one tile wait on every writer - use separate tiles.
 - tensor_tensor with BOTH operands in PSUM is not allowed; ACT table
   loads are inserted lazily before the first user of each table.
 - tensor_tensor_reduce / custom DVE ops hard-crash this runtime - avoid.
 - PE matmul starts are strictly pc-monotone: a semaphore-stalled matmul
   blocks every later one, so order matmuls by data-arrival time.
"""

import numpy as np

import concourse.bacc as bacc
import concourse.mybir as mybir
import concourse.tile as tile
from concourse.bass_utils import run_bass_kernel_spmd

N_CORES = 8
B, C, H, W_ = 32, 8, 24, 24
R = H * W_          # 576 routes
KJ = 128            # fused (j=16, k=8) W column axis, j-major
D = 16
NB = B // N_CORES   # 4 batches per core
WX = KJ + C         # 136 cols per (batch, tile)
FULL_T = 4          # full 128-row r-tiles per batch
BCOLS = FULL_T * WX                  # 544 cols per batch (full tiles)
# col layout: b0 | b1 | h01 | mask | h23 | b3d3 | b2 | b3d0-2.
# b3's last tile and the halves ride the steadier sync queue so the
# jittery swdge queue carries less of the chain-gating data.
B0, B1 = 0, BCOLS
H01 = 2 * BCOLS                      # 1088
MASK_OFF = H01 + WX                  # 1224
H23 = MASK_OFF + KJ                  # 1352
B3D3 = H23 + WX                      # 1488
B2 = B3D3 + WX                       # 1624
B3A = B2 + BCOLS                     # 2168 (b3 d0..d2)
COLS = B3A + 3 * WX                  # 2576
T3COLS = [B3A, B3A + WX, B3A + 2 * WX, B3D3]
DMA1_END = B2                        # cols [0, 1624): sync HWDGE, 416KB
RNORM = 1.0 / float(R)
RNORM2 = RNORM * RNORM
RNORM3 = RNORM2 * RNORM
N_JUNK = 8

_cached_nc = None
_last_in_maps = None


def _build():
    nc = bacc.Bacc(trn_type="TRN2")
    f32 = mybir.dt.float32
    f16 = mybir.dt.float16

    wx_h = nc.dram_tensor("wx", [128, COLS], f16, kind="ExternalInput")
    out_h = nc.dram_tensor("out", [NB, D], f32, kind="ExternalOutput")

    with tile.TileContext(nc) as tc:
        with (
            tc.tile_pool(name="sb", bufs=1) as sb,
            tc.tile_pool(name="gps", bufs=1, space="PSUM") as gps,
            tc.tile_pool(name="tps", bufs=1, space="PSUM") as tps,
            tc.tile_pool(name="jps", bufs=1, space="PSUM") as jps,
        ):
            # --- early consts / scratch (all off the critical path) ---
            oneh = sb.tile([128, NB], f16)
            nc.vector.memset(oneh[:], 0.0)
            for n in range(NB):
                nc.vector.memset(oneh[32 * n:32 * n + 8, n:n + 1], 1.0)
            eps_t = sb.tile([NB, 1], f32)
            nc.vector.memset(eps_t[:], 1e-8)
            # junk moving operand deliberately left uninitialized (garbage
            # fp16 is fine, results are discarded) - a full memset would
            # delay the first junk matmul by ~0.6us and the warm-up window
            # is tight.  Tile requires at least one writer per tile, so
            # memset only the 8 stationary columns.
            junk_sb = sb.tile([128, 520], f16)
            nc.vector.memset(junk_sb[:, 512:520], 0.5)
            # dummy Sqrt: forces the ACT table loads to execute here (~8us,
            # overlapping the DMA wait) instead of right before the real Sqrt
            dummy = sb.tile([NB, 1], f32)
            nc.scalar.activation(
                dummy[:], eps_t[:], mybir.ActivationFunctionType.Sqrt)

            # G in two PSUM tiles (b0/b1 and b2/b3) so the mask-multiply for
            # the first pair is not dependency-chained behind the second
            # pair's matmuls (Tile tracks PSUM regions by free-dim ranges
            # only, so partition-sliced reads of one tile wait on every
            # writer).  Zero the unused partition rows once so full-tile
            # reads see finite values (stale PSUM bits could be NaN).
            g_lo = gps.tile([64, KJ], f32, tag="g_lo")
            g_hi = gps.tile([64, KJ], f32, tag="g_hi")
            nc.vector.memset(g_lo[:], 0.0)
            nc.vector.memset(g_hi[:], 0.0)

            # --- PE warm-up across the DMA wait ---
            junk_ps = jps.tile([8, 512], f32)
            for _ in range(N_JUNK):
                nc.tensor.matmul(
                    junk_ps[:], junk_sb[:, 512:520], junk_sb[:, 0:512],
                    start=True, stop=True,
                )

            # --- input: two big DMAs on the two HWDGE queues ---
            # second DMA on the gpsimd SWDGE: its descriptor generation is
            # independent of the HWDGE ring (observed: two HWDGE-queue DMAs
            # start draining ~1.1us apart - generation is serialized)
            wx_t = sb.tile([128, COLS], f16)
            nc.sync.dma_start(wx_t[:, 0:DMA1_END], wx_h[:, 0:DMA1_END])
            nc.gpsimd.dma_start(wx_t[:, DMA1_END:COLS], wx_h[:, DMA1_END:COLS])

            mask16 = wx_t[:, MASK_OFF:MASK_OFF + KJ]

            # --- G[32n+k, j*8+k'] = sum_r x2[n, r, k] * W[n, r, j*8+k'] ---
            # batches 0/1 first (their DMA lands first); d-major within the
            # pair so the two column-group chains overlap on the array.
            # PE matmul starts are pc-monotone, so emission follows
            # data-arrival order (all sync-gated before swdge-gated).
            def mm(g_pair, col, c0, p0=0, nrow=128, start=False, stop=False):
                nc.tensor.matmul(
                    g_pair[32 * col:32 * col + 8, :],
                    wx_t[p0:p0 + nrow, c0 + KJ:c0 + WX],
                    wx_t[p0:p0 + nrow, c0:c0 + KJ],
                    start=start, stop=stop, tile_position=(p0, 32 * col),
                )

            for d in range(FULL_T):
                mm(g_lo, 0, B0 + d * WX, start=(d == 0))
                mm(g_lo, 1, B1 + d * WX, start=(d == 0))
            mm(g_lo, 0, H01, p0=0, nrow=64, stop=True)
            mm(g_lo, 1, H01, p0=64, nrow=64, stop=True)
            for d in range(FULL_T):
                mm(g_hi, 0, B2 + d * WX, start=(d == 0))
                mm(g_hi, 1, T3COLS[d], start=(d == 0))
            mm(g_hi, 0, H23, p0=0, nrow=64, stop=True)
            mm(g_hi, 1, H23, p0=64, nrow=64, stop=True)

            # --- k-diagonal mask-mul: the b0/b1 half runs while the PE is
            # still contracting b2/b3 ---
            pm = sb.tile([128, KJ], f16)
            nc.vector.tensor_mul(pm[0:64, :], g_lo[:], mask16[0:64, :])
            nc.vector.tensor_mul(pm[64:128, :], g_hi[:], mask16[64:128, :])
            # T2[n, j*8+k] = sum_p oneh[p, n] * pm[p, j*8+k]; the k-sum then
            # happens on a tiny [4, 16, 8] reduce that lands T in SBUF (fp32
            # accumulation, no fp16 r1 tile, no PSUM->SBUF copy)
            t2_ps = tps.tile([NB, KJ], f32)
            nc.tensor.matmul(t2_ps[:], oneh[:], pm[:], start=True, stop=True)
            t_sb = sb.tile([NB, D], f32)
            nc.vector.reduce_sum(
                t_sb[:], t2_ps[:].rearrange("p (j k) -> p j k", j=D),
                axis=mybir.AxisListType.X,
            )

            # --- squash: v = T * (normT/576^3) / ((1+norm) sqrt(norm+eps)),
            #     norm = normT/576^2,  normT = sum_j T^2 ---
            # T^2 and its row-sum (normT) in one DVE op
            sq = sb.tile([NB, D], f32)
            normt = sb.tile([NB, 1], f32)
            nc.vector.scalar_tensor_tensor(
                out=sq[:], in0=t_sb[:], scalar=1.0, in1=t_sb[:],
                op0=mybir.AluOpType.mult, op1=mybir.AluOpType.mult,
                accum_out=normt[:],
            )
            # 1/((1+norm)q) as two parallel reciprocals: 1/a1 folds into the
            # coefficient during the Sqrt's latency; only 1/q remains on the
            # post-Sqrt path.
            q = sb.tile([NB, 1], f32)
            nc.scalar.activation(
                q[:], normt[:], mybir.ActivationFunctionType.Sqrt,
                bias=eps_t[:], scale=RNORM2,
            )
            a1 = sb.tile([NB, 1], f32)
            nc.vector.tensor_scalar(
                out=a1[:], in0=normt[:], scalar1=RNORM2, scalar2=1.0,
                op0=mybir.AluOpType.mult, op1=mybir.AluOpType.add,
            )
            ra = sb.tile([NB, 1], f32)
            nc.vector.reciprocal(ra[:], a1[:])
            c2 = sb.tile([NB, 1], f32)
            nc.vector.tensor_scalar(
                out=c2[:], in0=normt[:], scalar1=RNORM3, scalar2=ra[:],
                op0=mybir.AluOpType.mult, op1=mybir.AluOpType.mult,
            )
            rq = sb.tile([NB, 1], f32)
            nc.vector.reciprocal(rq[:], q[:])
            v_sb = sb.tile([NB, D], f32)
            nc.vector.tensor_scalar(
                out=v_sb[:], in0=t_sb[:], scalar1=c2[:], scalar2=rq[:],
                op0=mybir.AluOpType.mult, op1=mybir.AluOpType.mult,
            )

            nc.sync.dma_start(out_h[:, :], v_sb[:])

    nc.finalize()
    return nc


def _pack_inputs(x, w):
    x = np.ascontiguousarray(np.asarray(x), dtype=np.float32)
    w = np.ascontiguousarray(np.asarray(w), dtype=np.float32)
    x2 = x.reshape(B, C, R).transpose(0, 2, 1)          # [B, R, 8]
    # j-major column packing: wf[b, r, j*8+k] = W[b, r, k, j]
    wf = w.reshape(B, R, C, D).transpose(0, 1, 3, 2).reshape(B, R, KJ)
    blk = np.concatenate([wf, x2], axis=2)              # [B, 576, 136]
    full = (
        blk[:, :512].reshape(B, FULL_T, 128, WX).transpose(0, 1, 2, 3)
    )                                                   # [B, 4, 128, 136]
    half = blk[:, 512:]                                 # [B, 64, 136]
    p = np.arange(128)[:, None]
    c = np.arange(KJ)[None, :]
    mask = (((p % 32) < 8) & ((c % 8) == (p % 32))).astype(np.float32)
    in_maps = []
    for core in range(N_CORES):
        bs = [core * NB + n for n in range(NB)]
        wx = np.empty((128, COLS), np.float32)
        for d in range(FULL_T):
            wx[:, B0 + d * WX:B0 + (d + 1) * WX] = full[bs[0], d]
            wx[:, B1 + d * WX:B1 + (d + 1) * WX] = full[bs[1], d]
            wx[:, B2 + d * WX:B2 + (d + 1) * WX] = full[bs[2], d]
            wx[:, T3COLS[d]:T3COLS[d] + WX] = full[bs[3], d]
        wx[0:64, H01:H01 + WX] = half[bs[0]]
        wx[64:128, H01:H01 + WX] = half[bs[1]]
        wx[0:64, H23:H23 + WX] = half[bs[2]]
        wx[64:128, H23:H23 + WX] = half[bs[3]]
        wx[:, MASK_OFF:MASK_OFF + KJ] = mask
        in_maps.append({"wx": np.ascontiguousarray(wx.astype(np.float16))})
    return in_maps


def kernel(x, route_weights):
    global _cached_nc, _last_in_maps
    if _cached_nc is None:
        _cached_nc = _build()
    nc = _cached_nc

    in_maps = _pack_inputs(x, route_weights)
    _last_in_maps = in_maps

    res = run_bass_kernel_spmd(nc, in_maps, core_ids=list(range(N_CORES)))
    v = np.concatenate([r["out"] for r in res.results], axis=0)   # [32, 16]
    return np.ascontiguousarray(
        np.broadcast_to(v[:, None, :], (B, R, D)).astype(np.float32)
    )


# revision 30
# speedup vs baseline: 1.1041x; 1.1041x over previous
"""DigitCapsules routing kernel for 8 Trainium2 NeuronCores.

Math: in the reference, u_hat is an explicit broadcast of u_core over the
capsule axis i, so b stays constant along i in every routing iteration,
softmax over i is exactly uniform (1/K), and the whole 3-iteration routing
collapses (exactly, in floating point too) to:

    v[b, i, :] = squash((1/576) * sum_{r,k} x2[b, r, k] * W[b, r, k, :])

broadcast over i = 0..575, where x2 = x.reshape(B, 8, 576).transpose(0, 2, 1).
The i-broadcast is pure replication, done on the host after the gather
(np.broadcast_to) - the device computes and returns only the unique
[4, 16] v rows per core.

Sharding: batch dim B=32 across 8 cores, 4 batches per core (data parallel).

Device structure (single pass over one [128, 128] PSUM tile):
 - wx = [W | x2] packed fp16 on host: per batch 4 full r-tiles of 128 rows
   plus a half tile; two batches share each half tile (64 partitions each)
   so there is no zero padding.  A k-diagonal mask rides along (128 cols).
   Two big input DMAs on independent descriptor-generation paths: sync
   HWDGE and gpsimd SWDGE (two HWDGE-ring DMAs start draining ~1.1us
   apart - ring generation is serialized; SWDGE is not).
 - G: batch n's contraction lands in PSUM partitions 32n..32n+8 via PE
   column-group tiling, so the batches' accumulation chains overlap on the
   PE array.  Batches 0/1 are emitted first (their DMA lands first).
 - One mask-multiply per pair (fp16 out, partition-shifted DVE reads are
   fine) extracts the k-diagonal; a one-hot [128, 4] fp16 matmul gives
   T2 [4, 128] fp32, and a tiny [4, 16, 8] reduce lands T in SBUF.
 - Squash over [4, 16]: fused square+row-sum on DVE, Sqrt on ACT, then two
   PARALLEL reciprocals - 1/(1+norm) folds into the coefficient during the
   Sqrt latency (Rsqrt is banned for accuracy).  A dummy early Sqrt hoists
   the 1.3us ACT table load off the critical path.
 - Output: one 256-byte DMA of v [4, 16] fp32.
 - 8 N=512 junk matmuls (operands mostly uninitialized - Tile only needs
   one writer per tile) run during the input-DMA wait to warm the PE HAM
   clock gate (~3.4us of sustained activity flips PE from 1.2 to 2.4 GHz;
   warm matmuls pace at 56ns vs 107ns cold).

Perf notes from traces:
 - ~13.0us of the measured window is FIXED (measured with a do-nothing
   1-DMA-in/1-DMA-out kernel): bass init, tiny-DMA in/out latencies, and
   ~8us of teardown - a runtime-injected postamble clears all 254
   semaphores one instruction at a time.  Run-to-run spread is +-0.5us
   (HAM phase, SWDGE landing jitter); measured 17.5-18.2us typical vs
   the 20.4us baseline.
 - Input DMA sustains only ~20B/ns per SDMA engine at these sizes;
   engines 72-79 (the second SEngine half) are measurably slower.
 - tensor_tensor with BOTH operands in PSUM is not allowed; ACT table
   loads are inserted lazily before the first user of each table.
 - tensor_tensor_reduce / custom DVE ops hard-crash this runtime - avoid.
"""

import numpy as np

import concourse.bacc as bacc
import concourse.mybir as mybir
import concourse.tile as tile
from concourse.bass_utils import run_bass_kernel_spmd

N_CORES = 8
B, C, H, W_ = 32, 8, 24, 24
R = H * W_          # 576 routes
KJ = 128            # fused (j=16, k=8) W column axis, j-major
D = 16
NB = B // N_CORES   # 4 batches per core
WX = KJ + C         # 136 cols per (batch, tile)
FULL_T = 4          # full 128-row r-tiles per batch
BCOLS = FULL_T * WX                  # 544 cols per batch (full tiles)
# col layout: b0 | b1 | h01 | mask | b2 | b3 | h23
B0, B1 = 0, BCOLS
H01 = 2 * BCOLS                      # 1088
MASK_OFF = H01 + WX                  # 1224
B2 = MASK_OFF + KJ                   # 1352
B3 = B2 + BCOLS                      # 1896
H23 = B3 + BCOLS                     # 2440
COLS = H23 + WX                      # 2576
DMA1_END = B2                        # cols [0, 1352): b0, b1, h01, mask
RNORM = 1.0 / float(R)
RNORM2 = RNORM * RNORM
RNORM3 = RNORM2 * RNORM
N_JUNK = 8

_cached_nc = None
_last_in_maps = None


def _build():
    nc = bacc.Bacc(trn_type="TRN2")
    f32 = mybir.dt.float32
    f16 = mybir.dt.float16

    wx_h = nc.dram_tensor("wx", [128, COLS], f16, kind="ExternalInput")
    out_h = nc.dram_tensor("out", [NB, D], f32, kind="ExternalOutput")

    with tile.TileContext(nc) as tc:
        with (
            tc.tile_pool(name="sb", bufs=1) as sb,
            tc.tile_pool(name="gps", bufs=1, space="PSUM") as gps,
            tc.tile_pool(name="tps", bufs=1, space="PSUM") as tps,
            tc.tile_pool(name="jps", bufs=1, space="PSUM") as jps,
        ):
            # --- early consts / scratch (all off the critical path) ---
            oneh = sb.tile([128, NB], f16)
            nc.vector.memset(oneh[:], 0.0)
            for n in range(NB):
                nc.vector.memset(oneh[32 * n:32 * n + 8, n:n + 1], 1.0)
            eps_t = sb.tile([NB, 1], f32)
            nc.vector.memset(eps_t[:], 1e-8)
            # junk moving operand deliberately left uninitialized (garbage
            # fp16 is fine, results are discarded) - a full memset would
            # delay the first junk matmul by ~0.6us and the warm-up window
            # is tight.  Tile requires at least one writer per tile, so
            # memset only the 8 stationary columns.
            junk_sb = sb.tile([128, 520], f16)
            nc.vector.memset(junk_sb[:, 512:520], 0.5)
            # dummy Sqrt: forces the ACT table loads to execute here (~8us,
            # overlapping the DMA wait) instead of right before the real Sqrt
            dummy = sb.tile([NB, 1], f32)
            nc.scalar.activation(
                dummy[:], eps_t[:], mybir.ActivationFunctionType.Sqrt)

            # G in two PSUM tiles (b0/b1 and b2/b3) so the mask-multiply for
            # the first pair is not dependency-chained behind the second
            # pair's matmuls (Tile tracks PSUM regions by free-dim ranges
            # only, so partition-sliced reads of one tile wait on every
            # writer).  Zero the unused partition rows once so full-tile
            # reads see finite values (stale PSUM bits could be NaN).
            g_lo = gps.tile([64, KJ], f32, tag="g_lo")
            g_hi = gps.tile([64, KJ], f32, tag="g_hi")
            nc.vector.memset(g_lo[:], 0.0)
            nc.vector.memset(g_hi[:], 0.0)

            # --- PE warm-up across the DMA wait ---
            junk_ps = jps.tile([8, 512], f32)
            for _ in range(N_JUNK):
                nc.tensor.matmul(
                    junk_ps[:], junk_sb[:, 512:520], junk_sb[:, 0:512],
                    start=True, stop=True,
                )

            # --- input: two big DMAs on the two HWDGE queues ---
            # second DMA on the gpsimd SWDGE: its descriptor generation is
            # independent of the HWDGE ring (observed: two HWDGE-queue DMAs
            # start draining ~1.1us apart - generation is serialized)
            wx_t = sb.tile([128, COLS], f16)
            nc.sync.dma_start(wx_t[:, 0:DMA1_END], wx_h[:, 0:DMA1_END])
            nc.gpsimd.dma_start(wx_t[:, DMA1_END:COLS], wx_h[:, DMA1_END:COLS])

            mask16 = wx_t[:, MASK_OFF:MASK_OFF + KJ]

            # --- G[32n+k, j*8+k'] = sum_r x2[n, r, k] * W[n, r, j*8+k'] ---
            # batches 0/1 first (their DMA lands first); d-major within the
            # pair so the two column-group chains overlap on the array.
            def batch_mms(n, base, half_base, half_lo):
                mms = []
                for d in range(FULL_T):
                    c0 = base + d * WX
                    mms.append((n, wx_t[:, c0 + KJ:c0 + WX],
                                wx_t[:, c0:c0 + KJ], 0, False))
                p0 = 0 if half_lo else 64
                mms.append((n, wx_t[p0:p0 + 64, half_base + KJ:half_base + WX],
                            wx_t[p0:p0 + 64, half_base:half_base + KJ],
                            p0, True))
                return mms

            plan = [batch_mms(0, B0, H01, True), batch_mms(1, B1, H01, False),
                    batch_mms(2, B2, H23, True), batch_mms(3, B3, H23, False)]
            for g_pair, pair in ((g_lo, plan[0:2]), (g_hi, plan[2:4])):
                for d in range(FULL_T + 1):
                    for col, bm in enumerate(pair):
                        n, xap, wap, p0, last = bm[d]
                        nc.tensor.matmul(
                            g_pair[32 * col:32 * col + 8, :], xap, wap,
                            start=(d == 0), stop=last,
                            tile_position=(p0, 32 * col),
                        )

            # --- k-diagonal mask-mul: the b0/b1 half runs while the PE is
            # still contracting b2/b3 ---
            pm = sb.tile([128, KJ], f16)
            nc.vector.tensor_mul(pm[0:64, :], g_lo[:], mask16[0:64, :])
            nc.vector.tensor_mul(pm[64:128, :], g_hi[:], mask16[64:128, :])
            # T2[n, j*8+k] = sum_p oneh[p, n] * pm[p, j*8+k]; the k-sum then
            # happens on a tiny [4, 16, 8] reduce that lands T in SBUF (fp32
            # accumulation, no fp16 r1 tile, no PSUM->SBUF copy)
            t2_ps = tps.tile([NB, KJ], f32)
            nc.tensor.matmul(t2_ps[:], oneh[:], pm[:], start=True, stop=True)
            t_sb = sb.tile([NB, D], f32)
            nc.vector.reduce_sum(
                t_sb[:], t2_ps[:].rearrange("p (j k) -> p j k", j=D),
                axis=mybir.AxisListType.X,
            )

            # --- squash: v = T * (normT/576^3) / ((1+norm) sqrt(norm+eps)),
            #     norm = normT/576^2,  normT = sum_j T^2 ---
            # T^2 and its row-sum (normT) in one DVE op (the DVE
            # accumulator read costs ~70ns vs ACT's 277ns READ_ACCUMULATOR)
            sq = sb.tile([NB, D], f32)
            normt = sb.tile([NB, 1], f32)
            nc.vector.scalar_tensor_tensor(
                out=sq[:], in0=t_sb[:], scalar=1.0, in1=t_sb[:],
                op0=mybir.AluOpType.mult, op1=mybir.AluOpType.mult,
                accum_out=normt[:],
            )
            # 1/((1+norm)q) as two parallel reciprocals: 1/a1 folds into the
            # coefficient during the Sqrt's latency; only 1/q remains on the
            # post-Sqrt path.
            q = sb.tile([NB, 1], f32)
            nc.scalar.activation(
                q[:], normt[:], mybir.ActivationFunctionType.Sqrt,
                bias=eps_t[:], scale=RNORM2,
            )
            a1 = sb.tile([NB, 1], f32)
            nc.vector.tensor_scalar(
                out=a1[:], in0=normt[:], scalar1=RNORM2, scalar2=1.0,
                op0=mybir.AluOpType.mult, op1=mybir.AluOpType.add,
            )
            ra = sb.tile([NB, 1], f32)
            nc.vector.reciprocal(ra[:], a1[:])
            c2 = sb.tile([NB, 1], f32)
            nc.vector.tensor_scalar(
                out=c2[:], in0=normt[:], scalar1=RNORM3, scalar2=ra[:],
                op0=mybir.AluOpType.mult, op1=mybir.AluOpType.mult,
            )
            rq = sb.tile([NB, 1], f32)
            nc.vector.reciprocal(rq[:], q[:])
            v_sb = sb.tile([NB, D], f32)
            nc.vector.tensor_scalar(
                out=v_sb[:], in0=t_sb[:], scalar1=c2[:], scalar2=rq[:],
                op0=mybir.AluOpType.mult, op1=mybir.AluOpType.mult,
            )

            nc.sync.dma_start(out_h[:, :], v_sb[:])

    nc.finalize()
    return nc


def _pack_inputs(x, w):
    x = np.ascontiguousarray(np.asarray(x), dtype=np.float32)
    w = np.ascontiguousarray(np.asarray(w), dtype=np.float32)
    x2 = x.reshape(B, C, R).transpose(0, 2, 1)          # [B, R, 8]
    # j-major column packing: wf[b, r, j*8+k] = W[b, r, k, j]
    wf = w.reshape(B, R, C, D).transpose(0, 1, 3, 2).reshape(B, R, KJ)
    blk = np.concatenate([wf, x2], axis=2)              # [B, 576, 136]
    full = (
        blk[:, :512].reshape(B, FULL_T, 128, WX).transpose(0, 2, 1, 3)
        .reshape(B, 128, BCOLS)
    )                                                   # [B, 128, 544]
    half = blk[:, 512:]                                 # [B, 64, 136]
    p = np.arange(128)[:, None]
    c = np.arange(KJ)[None, :]
    mask = (((p % 32) < 8) & ((c % 8) == (p % 32))).astype(np.float32)
    in_maps = []
    for core in range(N_CORES):
        bs = [core * NB + n for n in range(NB)]
        wx = np.empty((128, COLS), np.float32)
        wx[:, B0:B0 + BCOLS] = full[bs[0]]
        wx[:, B1:B1 + BCOLS] = full[bs[1]]
        wx[0:64, H01:H01 + WX] = half[bs[0]]
        wx[64:128, H01:H01 + WX] = half[bs[1]]
        wx[:, MASK_OFF:MASK_OFF + KJ] = mask
        wx[:, B2:B2 + BCOLS] = full[bs[2]]
        wx[:, B3:B3 + BCOLS] = full[bs[3]]
        wx[0:64, H23:H23 + WX] = half[bs[2]]
        wx[64:128, H23:H23 + WX] = half[bs[3]]
        in_maps.append({"wx": np.ascontiguousarray(wx.astype(np.float16))})
    return in_maps


def kernel(x, route_weights):
    global _cached_nc, _last_in_maps
    if _cached_nc is None:
        _cached_nc = _build()
    nc = _cached_nc

    in_maps = _pack_inputs(x, route_weights)
    _last_in_maps = in_maps

    res = run_bass_kernel_spmd(nc, in_maps, core_ids=list(range(N_CORES)))
    v = np.concatenate([r["out"] for r in res.results], axis=0)   # [32, 16]
    return np.ascontiguousarray(
        np.broadcast_to(v[:, None, :], (B, R, D)).astype(np.float32)
    )
